# revision 74
# baseline (speedup 1.0000x reference)
"""Causal self-attention (B=2,T=2048,C=1024,H=16) on 8 trn2 NeuronCores.

Sharding: core c handles batch b=c//4 and 4 heads (c%4)*4..+4 (tensor-parallel
over heads x data-parallel over batch).

All matmuls in bf16 (inputs quantized host-side / on-engine; PSUM stays f32):
  stage A: qkT = (Wqk)^T @ x^T  (scale folded into Wq), V = x @ Wv (+ones col)
  stage B: per (head, kt-pair): S^T tile = K' Q -> exp(s-2) -> causal mask
           (tensor_mul with a precomputed triangular mask on diag tiles only)
  AV transposed: y[qt] += es_kt^T-slice @ V_kt  -> psY [128q, 4qt x 65]
           (65th col = ones -> l); y = psY * (1/l) per-partition -> bf16
  yT via DMA xbar transpose; proj: outT = Wp^T @ yT, PSUM -> DRAM direct.
Host sums the 4 per-batch partials, adds b_proj, transposes back.
"""
import sys

sys.path.insert(0, "/opt/trn_rl_repo")

import numpy as np
import ml_dtypes

import concourse.bass as bass
import concourse.mybir as mybir
import concourse.tile as tile
from concourse import bacc
from concourse.bass_utils import run_bass_kernel_spmd

B, T, C, H, HD = 2, 2048, 1024, 16, 64
NCORES = 8
HPC = 4            # heads per core
CT = C // 128      # 8 contraction tiles
TJ = T // 512      # 4 q chunks
TT = T // 128      # 16 tok tiles
VW = HPC * (HD + 1)  # 260: V cols per core incl. ones column per head
F32 = mybir.dt.float32
BF = mybir.dt.bfloat16
EXP = mybir.ActivationFunctionType.Exp

_CACHE = {}


def _emit(tc, nc, d):
    d_xT, d_wqk, d_wv, d_wp, d_bqk, d_bv, d_mega, d_out = d
    from contextlib import ExitStack
    with tc.tile_pool(name="const", bufs=1) as pc, \
         tc.tile_pool(name="qk", bufs=1) as pqk, \
         tc.tile_pool(name="vv", bufs=1) as pvv, \
         tc.tile_pool(name="yt", bufs=1) as pyt, \
         tc.tile_pool(name="w_in", bufs=1) as pw, \
         tc.tile_pool(name="x_in", bufs=1) as px, \
         tc.tile_pool(name="fill", bufs=2, space="PSUM") as pfill, \
         tc.tile_pool(name="ex", bufs=4) as pex, \
         tc.tile_pool(name="nrm", bufs=4) as pn, \
         tc.tile_pool(name="ysb", bufs=8) as pysb, \
         tc.tile_pool(name="po", bufs=4) as po:
        inner = ExitStack()
        psS = inner.enter_context(tc.tile_pool(name="psS", bufs=2, space="PSUM"))
        psY = inner.enter_context(tc.tile_pool(name="psY", bufs=2, space="PSUM"))
        bqk = pc.tile([128, 4], F32, tag="bqk")
        bv = pc.tile([128, VW], F32, tag="bv")
        mega = pc.tile([128, 896], BF, tag="mega")
        negtwo = pc.tile([128, 1], F32, tag="negtwo")
        warm = pc.tile([128, 512], BF, tag="warm")
        nc.gpsimd.memset(negtwo[:], -2.0)
        nc.gpsimd.memset(warm[:], 0.0)

        qkT = [pqk.tile([128, T], BF, tag=f"qk{i}", name=f"qkT{i}") for i in range(4)]
        V = [pvv.tile([128, VW], BF, tag=f"v{i}", name=f"V{i}") for i in range(TT)]
        yT = [pyt.tile([128, T], BF, tag=f"y{i}", name=f"yT{i}") for i in range(2)]
        wqk = pw.tile([128, CT * 512], BF, tag="wqk")
        wv = pw.tile([128, CT * VW], BF, tag="wv")
        wp = pc.tile([128, 2 * C], BF, tag="wp")
        xT = px.tile([128, CT * T], BF, tag="xT")

        # input DMAs: xT is tj-major [tj, ct, 512], wqk is mo-major
        # [mo, ct, 128] so the first qk group only needs 2 DMAs
        nc.sync.dma_start(wqk[:, :1024], d_wqk[:, :1024])
        nc.sync.dma_start(xT[:, :4096], d_xT[:, :4096])
        for mo in range(1, 4):
            nc.sync.dma_start(
                wqk[:, mo * 1024:(mo + 1) * 1024],
                d_wqk[:, mo * 1024:(mo + 1) * 1024])
        nc.sync.dma_start(bqk[:], d_bqk)
        nc.sync.dma_start(wv[:], d_wv)
        nc.sync.dma_start(bv[:], d_bv)
        nc.sync.dma_start(mega[:], d_mega)
        for tj in range(1, TJ):
            nc.sync.dma_start(
                xT[:, tj * 4096:(tj + 1) * 4096],
                d_xT[:, tj * 4096:(tj + 1) * 4096])
        nc.sync.dma_start(wp[:], d_wp)

        # PE p-state warmup during the input-DMA wait (results unused)
        ws = psS.tile([128, 2, 512], F32, tag="s", name="warms")
        for i in range(8):
            nc.tensor.matmul(ws[0:64, 0, :], warm[:, :64], warm[:, :],
                             start=True, stop=True)

        # ---------------- stage A blocks (emitted via filler queue) -------
        def a_qk(tj, mo):
            def emit():
                ps = pfill.tile([128, 512], F32, tag="fill", name=f"psqk{tj}_{mo}")
                for ct in range(CT):
                    nc.tensor.matmul(
                        ps[:],
                        wqk[:, mo * 1024 + ct * 128:mo * 1024 + (ct + 1) * 128],
                        xT[:, tj * 4096 + ct * 512:tj * 4096 + (ct + 1) * 512],
                        start=(ct == 0), stop=(ct == CT - 1))
                nc.vector.tensor_scalar_add(
                    qkT[mo][:, tj * 512:(tj + 1) * 512], ps[:], bqk[:, mo:mo + 1])
            return emit

        def a_v(tt):
            tj, ti = divmod(tt, 4)

            def emit():
                psv = pfill.tile([128, 512], F32, tag="fill", name=f"psv{tt}")
                for ct in range(CT):
                    nc.tensor.matmul(
                        psv[:, :VW],
                        xT[:, tj * 4096 + ct * 512 + ti * 128:
                           tj * 4096 + ct * 512 + (ti + 1) * 128],
                        wv[:, ct * VW:(ct + 1) * VW],
                        start=(ct == 0), stop=(ct == CT - 1))
                nc.vector.tensor_add(V[tt][:], psv[:, :VW], bv[:])
            return emit

        def proj(qj, mo, pool=None, ptag="fill", act_copy=False):
            def emit():
                pps = (pool or pfill).tile([128, 512], F32, tag=ptag,
                                           name=f"pps{qj}_{mo}")
                for kt2 in range(2):
                    nc.tensor.matmul(
                        pps[:],
                        wp[:, kt2 * C + mo * 128:kt2 * C + (mo + 1) * 128],
                        yT[kt2][:, qj * 512:(qj + 1) * 512],
                        start=(kt2 == 0), stop=(kt2 == 1))
                ot = po.tile([128, 512], BF, tag="ot")
                if act_copy:
                    nc.scalar.activation(
                        ot[:], pps[:], mybir.ActivationFunctionType.Copy)
                else:
                    nc.vector.tensor_copy(ot[:], pps[:])
                nc.sync.dma_start(
                    d_out[:, mo, qj * 512:(qj + 1) * 512], ot[:])
            return emit

        # stage A for tj=0 runs up front; the rest interleaves into attention
        for mo in range(4):
            a_qk(0, mo)()
        for tt in range(4):
            a_v(tt)()

        # -------- attention: one global pair pipeline across sections -----
        # units = (qj, hp, lh, p); score-matmuls for unit i+1 are emitted
        # before unit i's exp/AV so the PE never head-of-line blocks on the
        # Act engine, including across lh/hp/qj boundaries.
        units = []
        for qj, hp in [(0, 0), (0, 1), (1, 0), (1, 1),
                       (2, 0), (3, 0), (2, 1), (3, 1)]:
            for lh in range(2):
                for p in range(2 * qj + 2):
                    units.append((qj, hp, lh, p))

        # filler queue: ('A', tj) blocks must drain before any unit of
        # qj=tj and are otherwise held until qj >= tj-1 (don't front-load
        # PE work that later Act-bound windows will starve for); ('P', qj)
        # blocks are appended once yT[qj] is complete.
        fq = []
        for tj in range(1, TJ):
            for mo in range(4):
                fq.append(["A", tj, a_qk(tj, mo), 1707])
            for tt in range(4 * tj, 4 * tj + 4):
                fq.append(["A", tj, a_v(tt), 867])

        # debt-based pacing: filler is emitted only while the PE work emitted
        # so far trails the Act (exp) work emitted so far, so the PE never
        # races ahead and then stalls >3us (which would reset the PE p-state)
        clock = {"pe": 0.0, "act": 0.0}

        def drain_filler(cur_qj, need_tj=None, budget=False, force=0):
            i = 0
            n = 0
            while i < len(fq):
                kind, idx, fn, cost = fq[i][:4]
                forced = need_tj is not None and kind == "A" and idx <= need_tj
                if not forced:
                    if n >= force and (
                            not budget or clock["pe"] >= clock["act"] - 300):
                        break
                    if kind == "A" and idx > cur_qj + 1:
                        i += 1
                        continue
                fn()
                clock["pe"] += cost
                fq.pop(i)
                if not forced:
                    n += 1

        ysb_tiles = {}
        state = {}

        def sc(u):
            qj, hp, lh, p = u
            off = 256 if p == 2 * qj + 1 else 0
            s = psS.tile([128, 2, 512], F32, tag="s")
            es = pex.tile([128, 2, 512], BF, tag="es")
            for ki in range(2):
                kt = 2 * p + ki
                nc.tensor.matmul(
                    s[:, ki, off:512],
                    qkT[2 * hp + 1][64 * lh:64 * lh + 64, kt * 128:(kt + 1) * 128],
                    qkT[2 * hp][64 * lh:64 * lh + 64,
                                qj * 512 + off:(qj + 1) * 512],
                    start=True, stop=True)
            return s, es, off

        # distance to the next qj-crossing, to pre-spread the A-barrier
        nxt = [len(units)] * len(units)
        for i in range(len(units) - 2, -1, -1):
            nxt[i] = i + 1 if units[i + 1][0] != units[i][0] else nxt[i + 1]

        pend = {}
        for i, u in enumerate(units):
            qj, hp, lh, p = u
            npair = 2 * qj + 2
            h_loc = 2 * hp + lh
            if i == 0:
                drain_filler(qj, need_tj=qj)
                pend[0] = sc(u)
            if i + 1 < len(units):
                nqj = units[i + 1][0]
                if nqj > qj:
                    drain_filler(qj, need_tj=nqj)
                pend[i + 1] = sc(units[i + 1])
            s, es, off = pend.pop(i)
            nc.scalar.activation(
                es[:, :, off:512], s[:, :, off:512], EXP, bias=negtwo[:])
            clock["act"] += (1024 - 2 * off) * 0.833 + 185
            clock["pe"] += (512 - off) * 2 * 0.4167
            for ki in range(2):
                kt = 2 * p + ki
                r = kt - 4 * qj
                if r >= 0:
                    # causal mask: keep k<=q <-> p<=c-128r. Only cols
                    # [off, 128(r+1)) can contain k>q; masking just that
                    # range keeps clean AV qt-slices free of the mask dep.
                    mo_ = 384 - 128 * r
                    c1 = 128 * (r + 1)
                    sl = es[:, ki, off:c1]
                    eng = nc.vector if (kt % 2 == 0) else nc.gpsimd
                    eng.tensor_mul(sl, sl, mega[:, mo_ + off:mo_ + c1])
            # filler covers the exp->AV latency, paced by the act/pe debt;
            # near a qj-crossing, force remaining A-blocks out one per unit
            # so the barrier doesn't dump them in a lump
            near_cross = (nxt[i] - i <= 6 and nxt[i] < len(units)
                          and any(e[0] == "A" and e[1] <= units[nxt[i]][0]
                                  for e in fq))
            drain_filler(qj, budget=True,
                         force=1 if (near_cross or (qj, hp) == (3, 1)) else 0)
            # AV (transposed): psy[qt] += es_kt(qt-slice) @ V_kt
            key = (qj, hp, lh)
            if key not in state:
                state[key] = [psY.tile([128, 512], F32, tag="psy",
                                       name=f"psy{qj}_{hp}_{lh}"), True]
            psy, first_mm = state[key]
            for qt in range(4):
                for ki in range(2):
                    kt = 2 * p + ki
                    if kt > 4 * qj + qt:
                        continue
                    nc.tensor.matmul(
                        psy[:, qt * 128:qt * 128 + 65],
                        es[:, ki, qt * 128:(qt + 1) * 128],
                        V[kt][:, h_loc * 65:h_loc * 65 + 65],
                        start=first_mm,
                        stop=(kt == 4 * qj + qt),
                        skip_group_check=True)
                    first_mm = False
                    clock["pe"] += 27
            state[key][1] = first_mm
            if (qj, hp, lh) == (3, 1, 1) and p >= 6:
                # epilogue fast-path: normalize + transpose each qt as soon
                # as its AV accumulation stops so only qt2/qt3 trail the
                # final exp
                qts = (0, 1) if p == 6 else (2, 3)
                y_sb = ysb_tiles[qj]
                rc = pn.tile([128, 4], F32, tag="rc", name=f"rcE{p}")
                for qt in qts:
                    sb_q = pn.tile([128, 128], F32, tag="sbq", name=f"sbq{qt}")
                    nc.vector.tensor_copy(
                        sb_q[:], psy[:, qt * 128:(qt + 1) * 128])
                    nc.vector.reciprocal(rc[:, qt:qt + 1], sb_q[:, 64:65])
                    nc.gpsimd.tensor_scalar_mul(
                        y_sb[qt][:, h_loc * 64:h_loc * 64 + 64],
                        sb_q[:, 0:64], rc[:, qt:qt + 1])
                    nc.sync.dma_start_transpose(
                        yT[1][:, qj * 512 + qt * 128:qj * 512 + (qt + 1) * 128],
                        y_sb[qt][:, 128:256])
                continue
            if p != npair - 1:
                continue
            # last pair of this head: normalize y = psy * (1/l).
            # GPSIMD can't read PSUM: drain psy to SBUF once (DVE), then
            # reciprocal + per-head muls run off SBUF (Pool-legal).
            if qj not in ysb_tiles:
                ysb_tiles[qj] = [
                    pysb.tile([128, 256], BF, tag="ysb", name=f"ysb{qj}_{q}")
                    for q in range(4)]
            y_sb = ysb_tiles[qj]
            sb_y = pn.tile([128, 512], F32, tag="sby")
            nc.vector.tensor_copy(sb_y[:], psy[:])
            rc = pn.tile([128, 4], F32, tag="rc")
            for qt in range(4):
                nc.vector.reciprocal(
                    rc[:, qt:qt + 1], sb_y[:, qt * 128 + 64:qt * 128 + 65])
            for qt in range(4):
                nc.gpsimd.tensor_scalar_mul(
                    y_sb[qt][:, h_loc * 64:h_loc * 64 + 64],
                    sb_y[:, qt * 128:qt * 128 + 64],
                    rc[:, qt:qt + 1])
            if lh == 1:
                # both heads of this pair done: transpose to yT
                for qt in range(4):
                    nc.sync.dma_start_transpose(
                        yT[hp][:, qj * 512 + qt * 128:qj * 512 + (qt + 1) * 128],
                        y_sb[qt][:, hp * 128:(hp + 1) * 128])
                if hp == 1:
                    for mo in range(8):
                        fq.append(["P", qj, proj(qj, mo), 427, mo])
        # drain leftover filler inside the attention scope, then run proj(3)
        # through a wide PSUM ring (psS/psY banks released) so its 8 blocks
        # stream without ring stalls
        while fq:
            fq.pop(0)[2]()
        inner.close()
        # tail proj: copies split DVE/Act, outputs staged into one tile so a
        # single strided DMA replaces 8 serialized HWDGE generations
        with tc.tile_pool(name="tail", bufs=6, space="PSUM") as ptail:
            ot_mega = po.tile([128, 8, 512], BF, tag="otm", name="ot_mega")
            # qt01 columns of yT(3) finish one pair earlier than qt23 (the
            # epilogue transposes them at p==6), so for 6 of 8 mo blocks the
            # first-half matmuls pre-run during the final exp window
            pps_t = {}
            for mo in range(6):
                pps = ptail.tile([128, 512], F32, tag="tp", name=f"tp{mo}")
                pps_t[mo] = pps
                for kt2 in range(2):
                    nc.tensor.matmul(
                        pps[:, 0:256],
                        wp[:, kt2 * C + mo * 128:kt2 * C + (mo + 1) * 128],
                        yT[kt2][:, 3 * 512:3 * 512 + 256],
                        start=(kt2 == 0), stop=(kt2 == 1),
                        skip_group_check=True)
            for mo in range(8):
                if mo < 6:
                    pps = pps_t[mo]
                    for kt2 in range(2):
                        nc.tensor.matmul(
                            pps[:, 256:512],
                            wp[:, kt2 * C + mo * 128:kt2 * C + (mo + 1) * 128],
                            yT[kt2][:, 3 * 512 + 256:4 * 512],
                            start=False, stop=(kt2 == 1),
                            skip_group_check=True)
                else:
                    pps = ptail.tile([128, 512], F32, tag="tp", name=f"tp{mo}")
                    for kt2 in range(2):
                        nc.tensor.matmul(
                            pps[:],
                            wp[:, kt2 * C + mo * 128:kt2 * C + (mo + 1) * 128],
                            yT[kt2][:, 3 * 512:4 * 512],
                            start=(kt2 == 0), stop=(kt2 == 1))
                if mo % 2 == 1:
                    nc.scalar.activation(
                        ot_mega[:, mo, :], pps[:],
                        mybir.ActivationFunctionType.Copy)
                else:
                    nc.vector.tensor_copy(ot_mega[:, mo, :], pps[:])
            nc.sync.dma_start(d_out[:, :, 3 * 512:4 * 512], ot_mega[:])


def _build(reps=1):
    nc = bacc.Bacc("TRN2", target_bir_lowering=False, debug=False)
    d = (
        nc.dram_tensor("xT", [128, CT * T], BF, kind="ExternalInput").ap(),
        nc.dram_tensor("wqk", [128, CT * 512], BF, kind="ExternalInput").ap(),
        nc.dram_tensor("wv", [128, CT * VW], BF, kind="ExternalInput").ap(),
        nc.dram_tensor("wp", [128, 2 * C], BF, kind="ExternalInput").ap(),
        nc.dram_tensor("bqk", [128, 4], F32, kind="ExternalInput").ap(),
        nc.dram_tensor("bv", [128, VW], F32, kind="ExternalInput").ap(),
        nc.dram_tensor("mega", [128, 896], BF, kind="ExternalInput").ap(),
        nc.dram_tensor("outT", [128, 8, T], BF, kind="ExternalOutput").ap(),
    )
    with tile.TileContext(nc) as tc:
        for rep in range(reps):
            if rep:
                tc.strict_bb_all_engine_barrier()
            _emit(tc, nc, d)
    nc.compile()
    return nc


def _sb(a):
    """[128k, n] -> SBUF layout [128, k*n] (k-tile-major along free dim)."""
    k = a.shape[0] // 128
    return np.ascontiguousarray(
        a.reshape(k, 128, a.shape[1]).transpose(1, 0, 2).reshape(128, -1))


def _bf(a):
    return np.ascontiguousarray(a).astype(ml_dtypes.bfloat16)


def _prep_in_maps(inputs):
    x = np.asarray(inputs["x"], np.float32)
    W_attn = np.asarray(inputs["W_attn"], np.float32)
    b_attn = np.asarray(inputs["b_attn"], np.float32)
    W_proj = np.asarray(inputs["W_proj"], np.float32)

    scale = 1.0 / np.sqrt(HD)
    # mega[p, j] = 1 iff j >= p + 384  (causal mask slices)
    mega = (np.arange(896)[None, :] >= np.arange(128)[:, None] + 384)

    in_maps = []
    for c in range(NCORES):
        b, g = divmod(c, 4)
        heads = [4 * g + i for i in range(HPC)]
        xT = _sb(np.ascontiguousarray(x[b].T))                      # [128, 8*2048]
        # tj-major: [128, tj, ct, 512] so one DMA covers a whole tj chunk
        xT = np.ascontiguousarray(
            xT.reshape(128, CT, TJ, 512).transpose(0, 2, 1, 3).reshape(128, -1))

        wq = [W_attn[:, h * HD:(h + 1) * HD] * scale for h in heads]
        wk = [W_attn[:, C + h * HD:C + (h + 1) * HD] for h in heads]
        wqk = np.concatenate(
            [wq[0], wq[1], wk[0], wk[1], wq[2], wq[3], wk[2], wk[3]], axis=1)
        wqk = _sb(wqk)                                              # [128, 8*512]
        # mo-major: [128, mo, ct, 128]
        wqk = np.ascontiguousarray(
            wqk.reshape(128, CT, 4, 128).transpose(0, 2, 1, 3).reshape(128, -1))

        wv = np.zeros((C, VW), np.float32)
        for i, h in enumerate(heads):
            wv[:, i * 65:i * 65 + 64] = W_attn[:, 2 * C + h * HD:2 * C + (h + 1) * HD]
        wv = _sb(wv)                                                # [128, 8*260]

        wp = np.zeros((128, 2 * C), np.float32)
        for kt2 in range(2):
            rows = np.concatenate(
                [W_proj[heads[2 * kt2 + j] * HD:(heads[2 * kt2 + j] + 1) * HD, :]
                 for j in range(2)], axis=0)                        # [128, 1024]
            wp[:, kt2 * C:(kt2 + 1) * C] = rows

        bqk = np.zeros((128, 4), np.float32)
        for i2 in range(2):   # head pair
            for j in range(2):
                h = heads[2 * i2 + j]
                bqk[64 * j:64 * j + 64, 2 * i2] = b_attn[h * HD:(h + 1) * HD] * scale
                bqk[64 * j:64 * j + 64, 2 * i2 + 1] = b_attn[C + h * HD:C + (h + 1) * HD]

        bv = np.zeros(VW, np.float32)
        for i, h in enumerate(heads):
            bv[i * 65:i * 65 + 64] = b_attn[2 * C + h * HD:2 * C + (h + 1) * HD]
            bv[i * 65 + 64] = 1.0
        bv = np.tile(bv[None, :], (128, 1)).astype(np.float32)

        in_maps.append({"xT": _bf(xT), "wqk": _bf(wqk), "wv": _bf(wv),
                        "wp": _bf(wp), "bqk": bqk, "bv": bv,
                        "mega": _bf(mega)})
    return in_maps


def kernel(x, W_attn, b_attn, W_proj, b_proj):
    in_maps = _prep_in_maps(dict(x=x, W_attn=W_attn, b_attn=b_attn,
                                 W_proj=W_proj, b_proj=b_proj))
    if "nc" not in _CACHE:
        _CACHE["nc"] = _build()
    nc = _CACHE["nc"]
    res = run_bass_kernel_spmd(nc, in_maps, core_ids=list(range(NCORES)))

    out = np.zeros((B, T, C), np.float32)
    for c in range(NCORES):
        b = c // 4
        oT = np.asarray(res.results[c]["outT"], np.float32)         # [128, 8*2048]
        oT = oT.reshape(128, 8, T).transpose(1, 0, 2).reshape(C, T)  # [C, T]
        out[b] += oT.T
    out += np.asarray(b_proj, np.float32)[None, None, :]
    return out


# revision 75
# speedup vs baseline: 1.0012x; 1.0012x over previous
"""Causal self-attention (B=2,T=2048,C=1024,H=16) on 8 trn2 NeuronCores.

Sharding: core c handles batch b=c//4 and 4 heads (c%4)*4..+4 (tensor-parallel
over heads x data-parallel over batch).

All matmuls in bf16 (inputs quantized host-side / on-engine; PSUM stays f32):
  stage A: qkT = (Wqk)^T @ x^T  (scale folded into Wq), V = x @ Wv (+ones col)
  stage B: per (head, kt-pair): S^T tile = K' Q -> exp(s-2) -> causal mask
           (tensor_mul with a precomputed triangular mask on diag tiles only)
  AV transposed: y[qt] += es_kt^T-slice @ V_kt  -> psY [128q, 4qt x 65]
           (65th col = ones -> l); y = psY * (1/l) per-partition -> bf16
  yT via DMA xbar transpose; proj: outT = Wp^T @ yT, PSUM -> DRAM direct.
Host sums the 4 per-batch partials, adds b_proj, transposes back.
"""
import sys

sys.path.insert(0, "/opt/trn_rl_repo")

import numpy as np
import ml_dtypes

import concourse.bass as bass
import concourse.mybir as mybir
import concourse.tile as tile
from concourse import bacc
from concourse.bass_utils import run_bass_kernel_spmd

B, T, C, H, HD = 2, 2048, 1024, 16, 64
NCORES = 8
HPC = 4            # heads per core
CT = C // 128      # 8 contraction tiles
TJ = T // 512      # 4 q chunks
TT = T // 128      # 16 tok tiles
VW = HPC * (HD + 1)  # 260: V cols per core incl. ones column per head
F32 = mybir.dt.float32
BF = mybir.dt.bfloat16
EXP = mybir.ActivationFunctionType.Exp

_CACHE = {}


def _emit(tc, nc, d):
    d_xT, d_wqk, d_wv, d_wp, d_bqk, d_bv, d_mega, d_out = d
    from contextlib import ExitStack
    with tc.tile_pool(name="const", bufs=1) as pc, \
         tc.tile_pool(name="qk", bufs=1) as pqk, \
         tc.tile_pool(name="vv", bufs=1) as pvv, \
         tc.tile_pool(name="yt", bufs=1) as pyt, \
         tc.tile_pool(name="w_in", bufs=1) as pw, \
         tc.tile_pool(name="x_in", bufs=1) as px, \
         tc.tile_pool(name="fill", bufs=2, space="PSUM") as pfill, \
         tc.tile_pool(name="ex", bufs=4) as pex, \
         tc.tile_pool(name="nrm", bufs=4) as pn, \
         tc.tile_pool(name="ysb", bufs=8) as pysb, \
         tc.tile_pool(name="po", bufs=4) as po:
        inner = ExitStack()
        psS = inner.enter_context(tc.tile_pool(name="psS", bufs=2, space="PSUM"))
        psY = inner.enter_context(tc.tile_pool(name="psY", bufs=2, space="PSUM"))
        bqk = pc.tile([128, 4], F32, tag="bqk")
        bv = pc.tile([128, VW], F32, tag="bv")
        mega = pc.tile([128, 896], BF, tag="mega")
        negtwo = pc.tile([128, 1], F32, tag="negtwo")
        warm = pc.tile([128, 512], BF, tag="warm")
        nc.gpsimd.memset(negtwo[:], -2.0)
        nc.gpsimd.memset(warm[:], 0.0)
        # preload the Exp activation table during the input-DMA wait so the
        # first real exp doesn't pay the 1.3us table load
        wes = pex.tile([128, 2, 512], BF, tag="es", name="warmes")
        nc.scalar.activation(wes[0:1, 0, 0:1], negtwo[0:1, :], EXP, bias=0.0)

        qkT = [pqk.tile([128, T], BF, tag=f"qk{i}", name=f"qkT{i}") for i in range(4)]
        V = [pvv.tile([128, VW], BF, tag=f"v{i}", name=f"V{i}") for i in range(TT)]
        yT = [pyt.tile([128, T], BF, tag=f"y{i}", name=f"yT{i}") for i in range(2)]
        wqk = pw.tile([128, CT * 512], BF, tag="wqk")
        wv = pw.tile([128, CT * VW], BF, tag="wv")
        wp = pc.tile([128, 2 * C], BF, tag="wp")
        xT = px.tile([128, CT * T], BF, tag="xT")

        # input DMAs: xT is tj-major [tj, ct, 512], wqk is mo-major
        # [mo, ct, 128] so the first qk group only needs 2 DMAs
        nc.sync.dma_start(wqk[:, :1024], d_wqk[:, :1024])
        nc.sync.dma_start(xT[:, :4096], d_xT[:, :4096])
        for mo in range(1, 4):
            nc.sync.dma_start(
                wqk[:, mo * 1024:(mo + 1) * 1024],
                d_wqk[:, mo * 1024:(mo + 1) * 1024])
        nc.sync.dma_start(bqk[:], d_bqk)
        nc.sync.dma_start(wv[:], d_wv)
        nc.sync.dma_start(bv[:], d_bv)
        nc.sync.dma_start(mega[:], d_mega)
        for tj in range(1, TJ):
            nc.sync.dma_start(
                xT[:, tj * 4096:(tj + 1) * 4096],
                d_xT[:, tj * 4096:(tj + 1) * 4096])
        nc.sync.dma_start(wp[:], d_wp)

        # PE p-state warmup during the input-DMA wait (results unused)
        ws = psS.tile([128, 2, 512], F32, tag="s", name="warms")
        for i in range(8):
            nc.tensor.matmul(ws[0:64, 0, :], warm[:, :64], warm[:, :],
                             start=True, stop=True)

        # ---------------- stage A blocks (emitted via filler queue) -------
        def a_qk(tj, mo):
            def emit():
                ps = pfill.tile([128, 512], F32, tag="fill", name=f"psqk{tj}_{mo}")
                for ct in range(CT):
                    nc.tensor.matmul(
                        ps[:],
                        wqk[:, mo * 1024 + ct * 128:mo * 1024 + (ct + 1) * 128],
                        xT[:, tj * 4096 + ct * 512:tj * 4096 + (ct + 1) * 512],
                        start=(ct == 0), stop=(ct == CT - 1))
                nc.vector.tensor_scalar_add(
                    qkT[mo][:, tj * 512:(tj + 1) * 512], ps[:], bqk[:, mo:mo + 1])
            return emit

        def a_v(tt):
            tj, ti = divmod(tt, 4)

            def emit():
                psv = pfill.tile([128, 512], F32, tag="fill", name=f"psv{tt}")
                for ct in range(CT):
                    nc.tensor.matmul(
                        psv[:, :VW],
                        xT[:, tj * 4096 + ct * 512 + ti * 128:
                           tj * 4096 + ct * 512 + (ti + 1) * 128],
                        wv[:, ct * VW:(ct + 1) * VW],
                        start=(ct == 0), stop=(ct == CT - 1))
                nc.vector.tensor_add(V[tt][:], psv[:, :VW], bv[:])
            return emit

        def proj(qj, mo, pool=None, ptag="fill", act_copy=False):
            def emit():
                pps = (pool or pfill).tile([128, 512], F32, tag=ptag,
                                           name=f"pps{qj}_{mo}")
                for kt2 in range(2):
                    nc.tensor.matmul(
                        pps[:],
                        wp[:, kt2 * C + mo * 128:kt2 * C + (mo + 1) * 128],
                        yT[kt2][:, qj * 512:(qj + 1) * 512],
                        start=(kt2 == 0), stop=(kt2 == 1))
                ot = po.tile([128, 512], BF, tag="ot")
                if act_copy:
                    nc.scalar.activation(
                        ot[:], pps[:], mybir.ActivationFunctionType.Copy)
                else:
                    nc.vector.tensor_copy(ot[:], pps[:])
                nc.sync.dma_start(
                    d_out[:, mo, qj * 512:(qj + 1) * 512], ot[:])
            return emit

        # stage A for tj=0 runs up front; the rest interleaves into attention
        for mo in range(4):
            a_qk(0, mo)()
        for tt in range(4):
            a_v(tt)()

        # -------- attention: one global pair pipeline across sections -----
        # units = (qj, hp, lh, p); score-matmuls for unit i+1 are emitted
        # before unit i's exp/AV so the PE never head-of-line blocks on the
        # Act engine, including across lh/hp/qj boundaries.
        units = []
        for qj, hp in [(0, 0), (0, 1), (1, 0), (1, 1),
                       (2, 0), (3, 0), (2, 1), (3, 1)]:
            for lh in range(2):
                for p in range(2 * qj + 2):
                    units.append((qj, hp, lh, p))

        # filler queue: ('A', tj) blocks must drain before any unit of
        # qj=tj and are otherwise held until qj >= tj-1 (don't front-load
        # PE work that later Act-bound windows will starve for); ('P', qj)
        # blocks are appended once yT[qj] is complete.
        fq = []
        for tj in range(1, TJ):
            for mo in range(4):
                fq.append(["A", tj, a_qk(tj, mo), 1707])
            for tt in range(4 * tj, 4 * tj + 4):
                fq.append(["A", tj, a_v(tt), 867])

        # debt-based pacing: filler is emitted only while the PE work emitted
        # so far trails the Act (exp) work emitted so far, so the PE never
        # races ahead and then stalls >3us (which would reset the PE p-state)
        clock = {"pe": 0.0, "act": 0.0}

        def drain_filler(cur_qj, need_tj=None, budget=False, force=0):
            i = 0
            n = 0
            while i < len(fq):
                kind, idx, fn, cost = fq[i][:4]
                forced = need_tj is not None and kind == "A" and idx <= need_tj
                if not forced:
                    if n >= force and (
                            not budget or clock["pe"] >= clock["act"] - 300):
                        break
                    if kind == "A" and idx > cur_qj + 1:
                        i += 1
                        continue
                fn()
                clock["pe"] += cost
                fq.pop(i)
                if not forced:
                    n += 1

        ysb_tiles = {}
        state = {}

        def sc(u):
            qj, hp, lh, p = u
            off = 256 if p == 2 * qj + 1 else 0
            s = psS.tile([128, 2, 512], F32, tag="s")
            es = pex.tile([128, 2, 512], BF, tag="es")
            for ki in range(2):
                kt = 2 * p + ki
                nc.tensor.matmul(
                    s[:, ki, off:512],
                    qkT[2 * hp + 1][64 * lh:64 * lh + 64, kt * 128:(kt + 1) * 128],
                    qkT[2 * hp][64 * lh:64 * lh + 64,
                                qj * 512 + off:(qj + 1) * 512],
                    start=True, stop=True)
            return s, es, off

        # distance to the next qj-crossing, to pre-spread the A-barrier
        nxt = [len(units)] * len(units)
        for i in range(len(units) - 2, -1, -1):
            nxt[i] = i + 1 if units[i + 1][0] != units[i][0] else nxt[i + 1]

        pend = {}
        for i, u in enumerate(units):
            qj, hp, lh, p = u
            npair = 2 * qj + 2
            h_loc = 2 * hp + lh
            if i == 0:
                drain_filler(qj, need_tj=qj)
                pend[0] = sc(u)
            if i + 1 < len(units):
                nqj = units[i + 1][0]
                if nqj > qj:
                    drain_filler(qj, need_tj=nqj)
                pend[i + 1] = sc(units[i + 1])
            s, es, off = pend.pop(i)
            nc.scalar.activation(
                es[:, :, off:512], s[:, :, off:512], EXP, bias=negtwo[:])
            clock["act"] += (1024 - 2 * off) * 0.833 + 185
            clock["pe"] += (512 - off) * 2 * 0.4167
            for ki in range(2):
                kt = 2 * p + ki
                r = kt - 4 * qj
                if r >= 0:
                    # causal mask: keep k<=q <-> p<=c-128r. Only cols
                    # [off, 128(r+1)) can contain k>q; masking just that
                    # range keeps clean AV qt-slices free of the mask dep.
                    mo_ = 384 - 128 * r
                    c1 = 128 * (r + 1)
                    sl = es[:, ki, off:c1]
                    eng = nc.vector if (kt % 2 == 0) else nc.gpsimd
                    eng.tensor_mul(sl, sl, mega[:, mo_ + off:mo_ + c1])
            # filler covers the exp->AV latency, paced by the act/pe debt;
            # near a qj-crossing, force remaining A-blocks out one per unit
            # so the barrier doesn't dump them in a lump
            near_cross = (nxt[i] - i <= 6 and nxt[i] < len(units)
                          and any(e[0] == "A" and e[1] <= units[nxt[i]][0]
                                  for e in fq))
            drain_filler(qj, budget=True,
                         force=1 if (near_cross or (qj, hp) == (3, 1)) else 0)
            # AV (transposed): psy[qt] += es_kt(qt-slice) @ V_kt
            key = (qj, hp, lh)
            if key not in state:
                state[key] = [psY.tile([128, 512], F32, tag="psy",
                                       name=f"psy{qj}_{hp}_{lh}"), True]
            psy, first_mm = state[key]
            for qt in range(4):
                for ki in range(2):
                    kt = 2 * p + ki
                    if kt > 4 * qj + qt:
                        continue
                    nc.tensor.matmul(
                        psy[:, qt * 128:qt * 128 + 65],
                        es[:, ki, qt * 128:(qt + 1) * 128],
                        V[kt][:, h_loc * 65:h_loc * 65 + 65],
                        start=first_mm,
                        stop=(kt == 4 * qj + qt),
                        skip_group_check=True)
                    first_mm = False
                    clock["pe"] += 27
            state[key][1] = first_mm
            if (qj, hp, lh) == (3, 1, 1) and p >= 6:
                # epilogue fast-path: normalize + transpose each qt as soon
                # as its AV accumulation stops so only qt2/qt3 trail the
                # final exp
                qts = (0, 1) if p == 6 else (2, 3)
                y_sb = ysb_tiles[qj]
                rc = pn.tile([128, 4], F32, tag="rc", name=f"rcE{p}")
                for qt in qts:
                    sb_q = pn.tile([128, 128], F32, tag="sbq", name=f"sbq{qt}")
                    nc.vector.tensor_copy(
                        sb_q[:], psy[:, qt * 128:(qt + 1) * 128])
                    nc.vector.reciprocal(rc[:, qt:qt + 1], sb_q[:, 64:65])
                    nc.gpsimd.tensor_scalar_mul(
                        y_sb[qt][:, h_loc * 64:h_loc * 64 + 64],
                        sb_q[:, 0:64], rc[:, qt:qt + 1])
                    nc.sync.dma_start_transpose(
                        yT[1][:, qj * 512 + qt * 128:qj * 512 + (qt + 1) * 128],
                        y_sb[qt][:, 128:256])
                continue
            if p != npair - 1:
                continue
            # last pair of this head: normalize y = psy * (1/l).
            # GPSIMD can't read PSUM: drain psy to SBUF once (DVE), then
            # reciprocal + per-head muls run off SBUF (Pool-legal).
            if qj not in ysb_tiles:
                ysb_tiles[qj] = [
                    pysb.tile([128, 256], BF, tag="ysb", name=f"ysb{qj}_{q}")
                    for q in range(4)]
            y_sb = ysb_tiles[qj]
            sb_y = pn.tile([128, 512], F32, tag="sby")
            nc.vector.tensor_copy(sb_y[:], psy[:])
            rc = pn.tile([128, 4], F32, tag="rc")
            for qt in range(4):
                nc.vector.reciprocal(
                    rc[:, qt:qt + 1], sb_y[:, qt * 128 + 64:qt * 128 + 65])
            for qt in range(4):
                nc.gpsimd.tensor_scalar_mul(
                    y_sb[qt][:, h_loc * 64:h_loc * 64 + 64],
                    sb_y[:, qt * 128:qt * 128 + 64],
                    rc[:, qt:qt + 1])
            if lh == 1:
                # both heads of this pair done: transpose to yT
                for qt in range(4):
                    nc.sync.dma_start_transpose(
                        yT[hp][:, qj * 512 + qt * 128:qj * 512 + (qt + 1) * 128],
                        y_sb[qt][:, hp * 128:(hp + 1) * 128])
                if hp == 1:
                    for mo in range(8):
                        fq.append(["P", qj, proj(qj, mo), 427, mo])
        # drain leftover filler inside the attention scope, then run proj(3)
        # through a wide PSUM ring (psS/psY banks released) so its 8 blocks
        # stream without ring stalls
        while fq:
            fq.pop(0)[2]()
        inner.close()
        # tail proj: copies split DVE/Act, outputs staged into one tile so a
        # single strided DMA replaces 8 serialized HWDGE generations
        with tc.tile_pool(name="tail", bufs=6, space="PSUM") as ptail:
            ot_mega = po.tile([128, 8, 512], BF, tag="otm", name="ot_mega")
            # qt01 columns of yT(3) finish one pair earlier than qt23 (the
            # epilogue transposes them at p==6), so for 6 of 8 mo blocks the
            # first-half matmuls pre-run during the final exp window
            pps_t = {}
            for mo in range(6):
                pps = ptail.tile([128, 512], F32, tag="tp", name=f"tp{mo}")
                pps_t[mo] = pps
                for kt2 in range(2):
                    nc.tensor.matmul(
                        pps[:, 0:256],
                        wp[:, kt2 * C + mo * 128:kt2 * C + (mo + 1) * 128],
                        yT[kt2][:, 3 * 512:3 * 512 + 256],
                        start=(kt2 == 0), stop=(kt2 == 1),
                        skip_group_check=True)
            for mo in range(8):
                if mo < 6:
                    pps = pps_t[mo]
                    for kt2 in range(2):
                        nc.tensor.matmul(
                            pps[:, 256:512],
                            wp[:, kt2 * C + mo * 128:kt2 * C + (mo + 1) * 128],
                            yT[kt2][:, 3 * 512 + 256:4 * 512],
                            start=False, stop=(kt2 == 1),
                            skip_group_check=True)
                else:
                    pps = ptail.tile([128, 512], F32, tag="tp", name=f"tp{mo}")
                    for kt2 in range(2):
                        nc.tensor.matmul(
                            pps[:],
                            wp[:, kt2 * C + mo * 128:kt2 * C + (mo + 1) * 128],
                            yT[kt2][:, 3 * 512:4 * 512],
                            start=(kt2 == 0), stop=(kt2 == 1))
                if mo % 2 == 1:
                    nc.scalar.activation(
                        ot_mega[:, mo, :], pps[:],
                        mybir.ActivationFunctionType.Copy)
                else:
                    nc.vector.tensor_copy(ot_mega[:, mo, :], pps[:])
            nc.sync.dma_start(d_out[:, :, 3 * 512:4 * 512], ot_mega[:])


def _build(reps=1):
    nc = bacc.Bacc("TRN2", target_bir_lowering=False, debug=False)
    d = (
        nc.dram_tensor("xT", [128, CT * T], BF, kind="ExternalInput").ap(),
        nc.dram_tensor("wqk", [128, CT * 512], BF, kind="ExternalInput").ap(),
        nc.dram_tensor("wv", [128, CT * VW], BF, kind="ExternalInput").ap(),
        nc.dram_tensor("wp", [128, 2 * C], BF, kind="ExternalInput").ap(),
        nc.dram_tensor("bqk", [128, 4], F32, kind="ExternalInput").ap(),
        nc.dram_tensor("bv", [128, VW], F32, kind="ExternalInput").ap(),
        nc.dram_tensor("mega", [128, 896], BF, kind="ExternalInput").ap(),
        nc.dram_tensor("outT", [128, 8, T], BF, kind="ExternalOutput").ap(),
    )
    with tile.TileContext(nc) as tc:
        for rep in range(reps):
            if rep:
                tc.strict_bb_all_engine_barrier()
            _emit(tc, nc, d)
    nc.compile()
    return nc


def _sb(a):
    """[128k, n] -> SBUF layout [128, k*n] (k-tile-major along free dim)."""
    k = a.shape[0] // 128
    return np.ascontiguousarray(
        a.reshape(k, 128, a.shape[1]).transpose(1, 0, 2).reshape(128, -1))


def _bf(a):
    return np.ascontiguousarray(a).astype(ml_dtypes.bfloat16)


def _prep_in_maps(inputs):
    x = np.asarray(inputs["x"], np.float32)
    W_attn = np.asarray(inputs["W_attn"], np.float32)
    b_attn = np.asarray(inputs["b_attn"], np.float32)
    W_proj = np.asarray(inputs["W_proj"], np.float32)

    scale = 1.0 / np.sqrt(HD)
    # mega[p, j] = 1 iff j >= p + 384  (causal mask slices)
    mega = (np.arange(896)[None, :] >= np.arange(128)[:, None] + 384)

    in_maps = []
    for c in range(NCORES):
        b, g = divmod(c, 4)
        heads = [4 * g + i for i in range(HPC)]
        xT = _sb(np.ascontiguousarray(x[b].T))                      # [128, 8*2048]
        # tj-major: [128, tj, ct, 512] so one DMA covers a whole tj chunk
        xT = np.ascontiguousarray(
            xT.reshape(128, CT, TJ, 512).transpose(0, 2, 1, 3).reshape(128, -1))

        wq = [W_attn[:, h * HD:(h + 1) * HD] * scale for h in heads]
        wk = [W_attn[:, C + h * HD:C + (h + 1) * HD] for h in heads]
        wqk = np.concatenate(
            [wq[0], wq[1], wk[0], wk[1], wq[2], wq[3], wk[2], wk[3]], axis=1)
        wqk = _sb(wqk)                                              # [128, 8*512]
        # mo-major: [128, mo, ct, 128]
        wqk = np.ascontiguousarray(
            wqk.reshape(128, CT, 4, 128).transpose(0, 2, 1, 3).reshape(128, -1))

        wv = np.zeros((C, VW), np.float32)
        for i, h in enumerate(heads):
            wv[:, i * 65:i * 65 + 64] = W_attn[:, 2 * C + h * HD:2 * C + (h + 1) * HD]
        wv = _sb(wv)                                                # [128, 8*260]

        wp = np.zeros((128, 2 * C), np.float32)
        for kt2 in range(2):
            rows = np.concatenate(
                [W_proj[heads[2 * kt2 + j] * HD:(heads[2 * kt2 + j] + 1) * HD, :]
                 for j in range(2)], axis=0)                        # [128, 1024]
            wp[:, kt2 * C:(kt2 + 1) * C] = rows

        bqk = np.zeros((128, 4), np.float32)
        for i2 in range(2):   # head pair
            for j in range(2):
                h = heads[2 * i2 + j]
                bqk[64 * j:64 * j + 64, 2 * i2] = b_attn[h * HD:(h + 1) * HD] * scale
                bqk[64 * j:64 * j + 64, 2 * i2 + 1] = b_attn[C + h * HD:C + (h + 1) * HD]

        bv = np.zeros(VW, np.float32)
        for i, h in enumerate(heads):
            bv[i * 65:i * 65 + 64] = b_attn[2 * C + h * HD:2 * C + (h + 1) * HD]
            bv[i * 65 + 64] = 1.0
        bv = np.tile(bv[None, :], (128, 1)).astype(np.float32)

        in_maps.append({"xT": _bf(xT), "wqk": _bf(wqk), "wv": _bf(wv),
                        "wp": _bf(wp), "bqk": bqk, "bv": bv,
                        "mega": _bf(mega)})
    return in_maps


def kernel(x, W_attn, b_attn, W_proj, b_proj):
    in_maps = _prep_in_maps(dict(x=x, W_attn=W_attn, b_attn=b_attn,
                                 W_proj=W_proj, b_proj=b_proj))
    if "nc" not in _CACHE:
        _CACHE["nc"] = _build()
    nc = _CACHE["nc"]
    res = run_bass_kernel_spmd(nc, in_maps, core_ids=list(range(NCORES)))

    out = np.zeros((B, T, C), np.float32)
    for c in range(NCORES):
        b = c // 4
        oT = np.asarray(res.results[c]["outT"], np.float32)         # [128, 8*2048]
        oT = oT.reshape(128, 8, T).transpose(1, 0, 2).reshape(C, T)  # [C, T]
        out[b] += oT.T
    out += np.asarray(b_proj, np.float32)[None, None, :]
    return out


# revision 76
# speedup vs baseline: 1.0061x; 1.0049x over previous
"""Causal self-attention (B=2,T=2048,C=1024,H=16) on 8 trn2 NeuronCores.

Sharding: core c handles batch b=c//4 and 4 heads (c%4)*4..+4 (tensor-parallel
over heads x data-parallel over batch).

All matmuls in bf16 (inputs quantized host-side / on-engine; PSUM stays f32):
  stage A: qkT = (Wqk)^T @ x^T  (scale folded into Wq), V = x @ Wv (+ones col)
  stage B: per (head, kt-pair): S^T tile = K' Q -> exp(s-2) -> causal mask
           (tensor_mul with a precomputed triangular mask on diag tiles only)
  AV transposed: y[qt] += es_kt^T-slice @ V_kt  -> psY [128q, 4qt x 65]
           (65th col = ones -> l); y = psY * (1/l) per-partition -> bf16
  yT via DMA xbar transpose; proj: outT = Wp^T @ yT, PSUM -> DRAM direct.
Host sums the 4 per-batch partials, adds b_proj, transposes back.
"""
import sys

sys.path.insert(0, "/opt/trn_rl_repo")

import numpy as np
import ml_dtypes

import concourse.bass as bass
import concourse.mybir as mybir
import concourse.tile as tile
from concourse import bacc
from concourse.bass_utils import run_bass_kernel_spmd

B, T, C, H, HD = 2, 2048, 1024, 16, 64
NCORES = 8
HPC = 4            # heads per core
CT = C // 128      # 8 contraction tiles
TJ = T // 512      # 4 q chunks
TT = T // 128      # 16 tok tiles
VW = HPC * (HD + 1)  # 260: V cols per core incl. ones column per head
F32 = mybir.dt.float32
BF = mybir.dt.bfloat16
EXP = mybir.ActivationFunctionType.Exp

_CACHE = {}


def _emit(tc, nc, d):
    d_xT, d_wqk, d_wv, d_wp, d_bqk, d_bv, d_mega, d_out = d
    from contextlib import ExitStack
    with tc.tile_pool(name="const", bufs=1) as pc, \
         tc.tile_pool(name="qk", bufs=1) as pqk, \
         tc.tile_pool(name="vv", bufs=1) as pvv, \
         tc.tile_pool(name="yt", bufs=1) as pyt, \
         tc.tile_pool(name="w_in", bufs=1) as pw, \
         tc.tile_pool(name="x_in", bufs=1) as px, \
         tc.tile_pool(name="fill", bufs=2, space="PSUM") as pfill, \
         tc.tile_pool(name="ex", bufs=4) as pex, \
         tc.tile_pool(name="nrm", bufs=4) as pn, \
         tc.tile_pool(name="ysb", bufs=8) as pysb, \
         tc.tile_pool(name="po", bufs=4) as po:
        inner = ExitStack()
        psS = inner.enter_context(tc.tile_pool(name="psS", bufs=2, space="PSUM"))
        psY = inner.enter_context(tc.tile_pool(name="psY", bufs=2, space="PSUM"))
        bqk = pc.tile([128, 4], F32, tag="bqk")
        bv = pc.tile([128, VW], F32, tag="bv")
        mega = pc.tile([128, 896], BF, tag="mega")
        negtwo = pc.tile([128, 1], F32, tag="negtwo")
        warm = pc.tile([128, 512], BF, tag="warm")
        nc.gpsimd.memset(negtwo[:], -2.0)
        nc.gpsimd.memset(warm[:], 0.0)
        # preload the Exp activation table during the input-DMA wait so the
        # first real exp doesn't pay the 1.3us table load
        wes = pex.tile([128, 2, 512], BF, tag="es", name="warmes")
        nc.scalar.activation(wes[0:1, 0, 0:1], negtwo[0:1, :], EXP, bias=0.0)

        qkT = [pqk.tile([128, T], BF, tag=f"qk{i}", name=f"qkT{i}") for i in range(4)]
        V = [pvv.tile([128, VW], BF, tag=f"v{i}", name=f"V{i}") for i in range(TT)]
        yT = [pyt.tile([128, T], BF, tag=f"y{i}", name=f"yT{i}") for i in range(2)]
        wqk = pw.tile([128, CT * 512], BF, tag="wqk")
        wv = pw.tile([128, CT * VW], BF, tag="wv")
        wp = pc.tile([128, 2 * C], BF, tag="wp")
        xT = px.tile([128, CT * T], BF, tag="xT")

        # input DMAs: xT is tj-major [tj, ct, 512], wqk is mo-major
        # [mo, ct, 128] so the first qk group only needs 2 DMAs
        nc.sync.dma_start(wqk[:, :1024], d_wqk[:, :1024])
        nc.sync.dma_start(xT[:, :4096], d_xT[:, :4096])
        for mo in range(1, 4):
            nc.sync.dma_start(
                wqk[:, mo * 1024:(mo + 1) * 1024],
                d_wqk[:, mo * 1024:(mo + 1) * 1024])
        nc.sync.dma_start(bqk[:], d_bqk)
        nc.sync.dma_start(wv[:], d_wv)
        nc.sync.dma_start(bv[:], d_bv)
        nc.sync.dma_start(mega[:], d_mega)
        for tj in range(1, TJ):
            nc.sync.dma_start(
                xT[:, tj * 4096:(tj + 1) * 4096],
                d_xT[:, tj * 4096:(tj + 1) * 4096])
        nc.sync.dma_start(wp[:], d_wp)

        # PE p-state warmup during the input-DMA wait (results unused)
        ws = psS.tile([128, 2, 512], F32, tag="s", name="warms")
        for i in range(8):
            nc.tensor.matmul(ws[0:64, 0, :], warm[:, :64], warm[:, :],
                             start=True, stop=True)

        # ---------------- stage A blocks (emitted via filler queue) -------
        def a_qk(tj, mo):
            def emit():
                ps = pfill.tile([128, 512], F32, tag="fill", name=f"psqk{tj}_{mo}")
                for ct in range(CT):
                    nc.tensor.matmul(
                        ps[:],
                        wqk[:, mo * 1024 + ct * 128:mo * 1024 + (ct + 1) * 128],
                        xT[:, tj * 4096 + ct * 512:tj * 4096 + (ct + 1) * 512],
                        start=(ct == 0), stop=(ct == CT - 1))
                nc.vector.tensor_scalar_add(
                    qkT[mo][:, tj * 512:(tj + 1) * 512], ps[:], bqk[:, mo:mo + 1])
            return emit

        def a_v(tt):
            tj, ti = divmod(tt, 4)

            def emit():
                psv = pfill.tile([128, 512], F32, tag="fill", name=f"psv{tt}")
                for ct in range(CT):
                    nc.tensor.matmul(
                        psv[:, :VW],
                        xT[:, tj * 4096 + ct * 512 + ti * 128:
                           tj * 4096 + ct * 512 + (ti + 1) * 128],
                        wv[:, ct * VW:(ct + 1) * VW],
                        start=(ct == 0), stop=(ct == CT - 1))
                nc.vector.tensor_add(V[tt][:], psv[:, :VW], bv[:])
            return emit

        def proj(qj, mo, pool=None, ptag="fill", act_copy=False):
            def emit():
                pps = (pool or pfill).tile([128, 512], F32, tag=ptag,
                                           name=f"pps{qj}_{mo}")
                for kt2 in range(2):
                    nc.tensor.matmul(
                        pps[:],
                        wp[:, kt2 * C + mo * 128:kt2 * C + (mo + 1) * 128],
                        yT[kt2][:, qj * 512:(qj + 1) * 512],
                        start=(kt2 == 0), stop=(kt2 == 1))
                ot = po.tile([128, 512], BF, tag="ot")
                if act_copy:
                    nc.scalar.activation(
                        ot[:], pps[:], mybir.ActivationFunctionType.Copy)
                else:
                    nc.vector.tensor_copy(ot[:], pps[:])
                nc.sync.dma_start(
                    d_out[:, mo, qj * 512:(qj + 1) * 512], ot[:])
            return emit

        # stage A for tj=0 runs up front; the rest interleaves into attention
        for mo in range(4):
            a_qk(0, mo)()
        for tt in range(4):
            a_v(tt)()

        # -------- attention: one global pair pipeline across sections -----
        # units = (qj, hp, lh, p); score-matmuls for unit i+1 are emitted
        # before unit i's exp/AV so the PE never head-of-line blocks on the
        # Act engine, including across lh/hp/qj boundaries.
        units = []
        for qj, hp in [(0, 0), (0, 1), (1, 0), (1, 1),
                       (2, 0), (3, 0), (2, 1), (3, 1)]:
            for lh in range(2):
                for p in range(2 * qj + 2):
                    units.append((qj, hp, lh, p))

        # filler queue: ('A', tj) blocks must drain before any unit of
        # qj=tj and are otherwise held until qj >= tj-1 (don't front-load
        # PE work that later Act-bound windows will starve for); ('P', qj)
        # blocks are appended once yT[qj] is complete.
        fq = []
        for tj in range(1, TJ):
            for mo in range(4):
                fq.append(["A", tj, a_qk(tj, mo), 1707])
            for tt in range(4 * tj, 4 * tj + 4):
                fq.append(["A", tj, a_v(tt), 867])

        # debt-based pacing: filler is emitted only while the PE work emitted
        # so far trails the Act (exp) work emitted so far, so the PE never
        # races ahead and then stalls >3us (which would reset the PE p-state)
        clock = {"pe": 0.0, "act": 0.0}

        def drain_filler(cur_qj, need_tj=None, budget=False, force=0):
            i = 0
            n = 0
            while i < len(fq):
                kind, idx, fn, cost = fq[i][:4]
                forced = need_tj is not None and kind == "A" and idx <= need_tj
                if not forced:
                    if n >= force and (
                            not budget or clock["pe"] >= clock["act"] - 300):
                        break
                    if kind == "A" and idx > cur_qj + 1:
                        i += 1
                        continue
                fn()
                clock["pe"] += cost
                fq.pop(i)
                if not forced:
                    n += 1

        ysb_tiles = {}
        state = {}

        def sc(u):
            qj, hp, lh, p = u
            off = 256 if p == 2 * qj + 1 else 0
            s = psS.tile([128, 2, 512], F32, tag="s")
            es = pex.tile([128, 2, 512], BF, tag="es")
            for ki in range(2):
                kt = 2 * p + ki
                nc.tensor.matmul(
                    s[:, ki, off:512],
                    qkT[2 * hp + 1][64 * lh:64 * lh + 64, kt * 128:(kt + 1) * 128],
                    qkT[2 * hp][64 * lh:64 * lh + 64,
                                qj * 512 + off:(qj + 1) * 512],
                    start=True, stop=True)
            return s, es, off

        # distance to the next qj-crossing, to pre-spread the A-barrier
        nxt = [len(units)] * len(units)
        for i in range(len(units) - 2, -1, -1):
            nxt[i] = i + 1 if units[i + 1][0] != units[i][0] else nxt[i + 1]

        pend = {}
        for i, u in enumerate(units):
            qj, hp, lh, p = u
            npair = 2 * qj + 2
            h_loc = 2 * hp + lh
            if i == 0:
                drain_filler(qj, need_tj=qj)
                pend[0] = sc(u)
            if i + 1 < len(units):
                nqj = units[i + 1][0]
                if nqj > qj:
                    drain_filler(qj, need_tj=nqj)
                pend[i + 1] = sc(units[i + 1])
            s, es, off = pend.pop(i)
            nc.scalar.activation(
                es[:, :, off:512], s[:, :, off:512], EXP, bias=negtwo[:])
            clock["act"] += (1024 - 2 * off) * 0.833 + 185
            clock["pe"] += (512 - off) * 2 * 0.4167
            for ki in range(2):
                kt = 2 * p + ki
                r = kt - 4 * qj
                if r >= 0:
                    # causal mask: keep k<=q <-> p<=c-128r. Only cols
                    # [off, 128(r+1)) can contain k>q; masking just that
                    # range keeps clean AV qt-slices free of the mask dep.
                    mo_ = 384 - 128 * r
                    c1 = 128 * (r + 1)
                    sl = es[:, ki, off:c1]
                    eng = nc.vector if (kt % 2 == 0) else nc.gpsimd
                    eng.tensor_mul(sl, sl, mega[:, mo_ + off:mo_ + c1])
            # filler covers the exp->AV latency, paced by the act/pe debt;
            # near a qj-crossing, force remaining A-blocks out one per unit
            # so the barrier doesn't dump them in a lump
            near_cross = (nxt[i] - i <= 6 and nxt[i] < len(units)
                          and any(e[0] == "A" and e[1] <= units[nxt[i]][0]
                                  for e in fq))
            drain_filler(qj, budget=True,
                         force=1 if (near_cross or (qj, hp) == (3, 1)) else 0)
            # AV (transposed): psy[qt] += es_kt(qt-slice) @ V_kt
            key = (qj, hp, lh)
            if key not in state:
                state[key] = [psY.tile([128, 512], F32, tag="psy",
                                       name=f"psy{qj}_{hp}_{lh}"), True]
            psy, first_mm = state[key]
            for qt in range(4):
                for ki in range(2):
                    kt = 2 * p + ki
                    if kt > 4 * qj + qt:
                        continue
                    nc.tensor.matmul(
                        psy[:, qt * 128:qt * 128 + 65],
                        es[:, ki, qt * 128:(qt + 1) * 128],
                        V[kt][:, h_loc * 65:h_loc * 65 + 65],
                        start=first_mm,
                        stop=(kt == 4 * qj + qt),
                        skip_group_check=True)
                    first_mm = False
                    clock["pe"] += 27
            state[key][1] = first_mm
            if (qj, hp, lh) == (3, 1, 1) and p >= 6:
                # epilogue fast-path: normalize + transpose each qt as soon
                # as its AV accumulation stops so only qt2/qt3 trail the
                # final exp
                qts = (0, 1) if p == 6 else (2, 3)
                y_sb = ysb_tiles[qj]
                rc = pn.tile([128, 4], F32, tag="rc", name=f"rcE{p}")
                for qt in qts:
                    sb_q = pn.tile([128, 128], F32, tag="sbq", name=f"sbq{qt}")
                    nc.vector.tensor_copy(
                        sb_q[:], psy[:, qt * 128:(qt + 1) * 128])
                    nc.vector.reciprocal(rc[:, qt:qt + 1], sb_q[:, 64:65])
                    nc.gpsimd.tensor_scalar_mul(
                        y_sb[qt][:, h_loc * 64:h_loc * 64 + 64],
                        sb_q[:, 0:64], rc[:, qt:qt + 1])
                    nc.sync.dma_start_transpose(
                        yT[1][:, qj * 512 + qt * 128:qj * 512 + (qt + 1) * 128],
                        y_sb[qt][:, 128:256])
                continue
            if p != npair - 1:
                continue
            # last pair of this head: normalize y = psy * (1/l).
            # GPSIMD can't read PSUM: drain psy to SBUF once (DVE), then
            # reciprocal + per-head muls run off SBUF (Pool-legal).
            if qj not in ysb_tiles:
                ysb_tiles[qj] = [
                    pysb.tile([128, 256], BF, tag="ysb", name=f"ysb{qj}_{q}")
                    for q in range(4)]
            y_sb = ysb_tiles[qj]
            sb_y = pn.tile([128, 512], F32, tag="sby")
            nc.vector.tensor_copy(sb_y[:], psy[:])
            rc = pn.tile([128, 4], F32, tag="rc")
            for qt in range(4):
                nc.vector.reciprocal(
                    rc[:, qt:qt + 1], sb_y[:, qt * 128 + 64:qt * 128 + 65])
            for qt in range(4):
                nc.gpsimd.tensor_scalar_mul(
                    y_sb[qt][:, h_loc * 64:h_loc * 64 + 64],
                    sb_y[:, qt * 128:qt * 128 + 64],
                    rc[:, qt:qt + 1])
            if lh == 1:
                # both heads of this pair done: transpose to yT
                for qt in range(4):
                    nc.sync.dma_start_transpose(
                        yT[hp][:, qj * 512 + qt * 128:qj * 512 + (qt + 1) * 128],
                        y_sb[qt][:, hp * 128:(hp + 1) * 128])
                if hp == 1:
                    for mo in range(8):
                        fq.append(["P", qj, proj(qj, mo), 427, mo])
        # drain leftover filler inside the attention scope, then run proj(3)
        # through a wide PSUM ring (psS/psY banks released) so its 8 blocks
        # stream without ring stalls
        while fq:
            fq.pop(0)[2]()
        inner.close()
        # tail proj: copies split DVE/Act, outputs staged into one tile so a
        # single strided DMA replaces 8 serialized HWDGE generations
        with tc.tile_pool(name="tail", bufs=6, space="PSUM") as ptail:
            ot_mega = po.tile([128, 8, 512], BF, tag="otm", name="ot_mega")
            # qt01 columns of yT(3) finish one pair earlier than qt23 (the
            # epilogue transposes them at p==6), so for 6 of 8 mo blocks the
            # first-half matmuls pre-run during the final exp window
            pps_t = {}
            for mo in range(6):
                pps = ptail.tile([128, 512], F32, tag="tp", name=f"tp{mo}")
                pps_t[mo] = pps
                for kt2 in range(2):
                    nc.tensor.matmul(
                        pps[:, 0:256],
                        wp[:, kt2 * C + mo * 128:kt2 * C + (mo + 1) * 128],
                        yT[kt2][:, 3 * 512:3 * 512 + 256],
                        start=(kt2 == 0), stop=(kt2 == 1),
                        skip_group_check=True)
            for mo in range(8):
                if mo < 6:
                    pps = pps_t[mo]
                    for kt2 in range(2):
                        nc.tensor.matmul(
                            pps[:, 256:512],
                            wp[:, kt2 * C + mo * 128:kt2 * C + (mo + 1) * 128],
                            yT[kt2][:, 3 * 512 + 256:4 * 512],
                            start=False, stop=(kt2 == 1),
                            skip_group_check=True)
                else:
                    pps = ptail.tile([128, 512], F32, tag="tp", name=f"tp{mo}")
                    for kt2 in range(2):
                        nc.tensor.matmul(
                            pps[:],
                            wp[:, kt2 * C + mo * 128:kt2 * C + (mo + 1) * 128],
                            yT[kt2][:, 3 * 512:4 * 512],
                            start=(kt2 == 0), stop=(kt2 == 1))
                if mo % 2 == 1:
                    nc.scalar.activation(
                        ot_mega[:, mo, :], pps[:],
                        mybir.ActivationFunctionType.Copy)
                else:
                    nc.vector.tensor_copy(ot_mega[:, mo, :], pps[:])
                # split the final DMA: the bulk streams out while the last
                # two blocks' copies finish, shortening the terminal chain
                if mo == 5:
                    nc.sync.dma_start(
                        d_out[:, 0:6, 3 * 512:4 * 512], ot_mega[:, 0:6, :])
            nc.sync.dma_start(
                d_out[:, 6:8, 3 * 512:4 * 512], ot_mega[:, 6:8, :])


def _build(reps=1):
    nc = bacc.Bacc("TRN2", target_bir_lowering=False, debug=False)
    d = (
        nc.dram_tensor("xT", [128, CT * T], BF, kind="ExternalInput").ap(),
        nc.dram_tensor("wqk", [128, CT * 512], BF, kind="ExternalInput").ap(),
        nc.dram_tensor("wv", [128, CT * VW], BF, kind="ExternalInput").ap(),
        nc.dram_tensor("wp", [128, 2 * C], BF, kind="ExternalInput").ap(),
        nc.dram_tensor("bqk", [128, 4], F32, kind="ExternalInput").ap(),
        nc.dram_tensor("bv", [128, VW], F32, kind="ExternalInput").ap(),
        nc.dram_tensor("mega", [128, 896], BF, kind="ExternalInput").ap(),
        nc.dram_tensor("outT", [128, 8, T], BF, kind="ExternalOutput").ap(),
    )
    with tile.TileContext(nc) as tc:
        for rep in range(reps):
            if rep:
                tc.strict_bb_all_engine_barrier()
            _emit(tc, nc, d)
    nc.compile()
    return nc


def _sb(a):
    """[128k, n] -> SBUF layout [128, k*n] (k-tile-major along free dim)."""
    k = a.shape[0] // 128
    return np.ascontiguousarray(
        a.reshape(k, 128, a.shape[1]).transpose(1, 0, 2).reshape(128, -1))


def _bf(a):
    return np.ascontiguousarray(a).astype(ml_dtypes.bfloat16)


def _prep_in_maps(inputs):
    x = np.asarray(inputs["x"], np.float32)
    W_attn = np.asarray(inputs["W_attn"], np.float32)
    b_attn = np.asarray(inputs["b_attn"], np.float32)
    W_proj = np.asarray(inputs["W_proj"], np.float32)

    scale = 1.0 / np.sqrt(HD)
    # mega[p, j] = 1 iff j >= p + 384  (causal mask slices)
    mega = (np.arange(896)[None, :] >= np.arange(128)[:, None] + 384)

    in_maps = []
    for c in range(NCORES):
        b, g = divmod(c, 4)
        heads = [4 * g + i for i in range(HPC)]
        xT = _sb(np.ascontiguousarray(x[b].T))                      # [128, 8*2048]
        # tj-major: [128, tj, ct, 512] so one DMA covers a whole tj chunk
        xT = np.ascontiguousarray(
            xT.reshape(128, CT, TJ, 512).transpose(0, 2, 1, 3).reshape(128, -1))

        wq = [W_attn[:, h * HD:(h + 1) * HD] * scale for h in heads]
        wk = [W_attn[:, C + h * HD:C + (h + 1) * HD] for h in heads]
        wqk = np.concatenate(
            [wq[0], wq[1], wk[0], wk[1], wq[2], wq[3], wk[2], wk[3]], axis=1)
        wqk = _sb(wqk)                                              # [128, 8*512]
        # mo-major: [128, mo, ct, 128]
        wqk = np.ascontiguousarray(
            wqk.reshape(128, CT, 4, 128).transpose(0, 2, 1, 3).reshape(128, -1))

        wv = np.zeros((C, VW), np.float32)
        for i, h in enumerate(heads):
            wv[:, i * 65:i * 65 + 64] = W_attn[:, 2 * C + h * HD:2 * C + (h + 1) * HD]
        wv = _sb(wv)                                                # [128, 8*260]

        wp = np.zeros((128, 2 * C), np.float32)
        for kt2 in range(2):
            rows = np.concatenate(
                [W_proj[heads[2 * kt2 + j] * HD:(heads[2 * kt2 + j] + 1) * HD, :]
                 for j in range(2)], axis=0)                        # [128, 1024]
            wp[:, kt2 * C:(kt2 + 1) * C] = rows

        bqk = np.zeros((128, 4), np.float32)
        for i2 in range(2):   # head pair
            for j in range(2):
                h = heads[2 * i2 + j]
                bqk[64 * j:64 * j + 64, 2 * i2] = b_attn[h * HD:(h + 1) * HD] * scale
                bqk[64 * j:64 * j + 64, 2 * i2 + 1] = b_attn[C + h * HD:C + (h + 1) * HD]

        bv = np.zeros(VW, np.float32)
        for i, h in enumerate(heads):
            bv[i * 65:i * 65 + 64] = b_attn[2 * C + h * HD:2 * C + (h + 1) * HD]
            bv[i * 65 + 64] = 1.0
        bv = np.tile(bv[None, :], (128, 1)).astype(np.float32)

        in_maps.append({"xT": _bf(xT), "wqk": _bf(wqk), "wv": _bf(wv),
                        "wp": _bf(wp), "bqk": bqk, "bv": bv,
                        "mega": _bf(mega)})
    return in_maps


def kernel(x, W_attn, b_attn, W_proj, b_proj):
    in_maps = _prep_in_maps(dict(x=x, W_attn=W_attn, b_attn=b_attn,
                                 W_proj=W_proj, b_proj=b_proj))
    if "nc" not in _CACHE:
        _CACHE["nc"] = _build()
    nc = _CACHE["nc"]
    res = run_bass_kernel_spmd(nc, in_maps, core_ids=list(range(NCORES)))

    out = np.zeros((B, T, C), np.float32)
    for c in range(NCORES):
        b = c // 4
        oT = np.asarray(res.results[c]["outT"], np.float32)         # [128, 8*2048]
        oT = oT.reshape(128, 8, T).transpose(1, 0, 2).reshape(C, T)  # [C, T]
        out[b] += oT.T
    out += np.asarray(b_proj, np.float32)[None, None, :]
    return out


# revision 77
# speedup vs baseline: 1.0065x; 1.0004x over previous
"""Causal self-attention (B=2,T=2048,C=1024,H=16) on 8 trn2 NeuronCores.

Sharding: core c handles batch b=c//4 and 4 heads (c%4)*4..+4 (tensor-parallel
over heads x data-parallel over batch).

All matmuls in bf16 (inputs quantized host-side / on-engine; PSUM stays f32):
  stage A: qkT = (Wqk)^T @ x^T  (scale folded into Wq), V = x @ Wv (+ones col)
  stage B: per (head, kt-pair): S^T tile = K' Q -> exp(s-2) -> causal mask
           (tensor_mul with a precomputed triangular mask on diag tiles only)
  AV transposed: y[qt] += es_kt^T-slice @ V_kt  -> psY [128q, 4qt x 65]
           (65th col = ones -> l); y = psY * (1/l) per-partition -> bf16
  yT via DMA xbar transpose; proj: outT = Wp^T @ yT, PSUM -> DRAM direct.
Host sums the 4 per-batch partials, adds b_proj, transposes back.
"""
import sys

sys.path.insert(0, "/opt/trn_rl_repo")

import numpy as np
import ml_dtypes

import concourse.bass as bass
import concourse.mybir as mybir
import concourse.tile as tile
from concourse import bacc
from concourse.bass_utils import run_bass_kernel_spmd

B, T, C, H, HD = 2, 2048, 1024, 16, 64
NCORES = 8
HPC = 4            # heads per core
CT = C // 128      # 8 contraction tiles
TJ = T // 512      # 4 q chunks
TT = T // 128      # 16 tok tiles
VW = HPC * (HD + 1)  # 260: V cols per core incl. ones column per head
F32 = mybir.dt.float32
BF = mybir.dt.bfloat16
EXP = mybir.ActivationFunctionType.Exp

_CACHE = {}


def _emit(tc, nc, d):
    d_xT, d_wqk, d_wv, d_wp, d_bqk, d_bv, d_mega, d_out = d
    from contextlib import ExitStack
    with tc.tile_pool(name="const", bufs=1) as pc, \
         tc.tile_pool(name="qk", bufs=1) as pqk, \
         tc.tile_pool(name="vv", bufs=1) as pvv, \
         tc.tile_pool(name="yt", bufs=1) as pyt, \
         tc.tile_pool(name="w_in", bufs=1) as pw, \
         tc.tile_pool(name="x_in", bufs=1) as px, \
         tc.tile_pool(name="fill", bufs=2, space="PSUM") as pfill, \
         tc.tile_pool(name="ex", bufs=4) as pex, \
         tc.tile_pool(name="nrm", bufs=4) as pn, \
         tc.tile_pool(name="ysb", bufs=8) as pysb, \
         tc.tile_pool(name="po", bufs=4) as po:
        inner = ExitStack()
        psS = inner.enter_context(tc.tile_pool(name="psS", bufs=2, space="PSUM"))
        psY = inner.enter_context(tc.tile_pool(name="psY", bufs=2, space="PSUM"))
        bqk = pc.tile([128, 4], F32, tag="bqk")
        bv = pc.tile([128, VW], F32, tag="bv")
        mega = pc.tile([128, 896], BF, tag="mega")
        negtwo = pc.tile([128, 1], F32, tag="negtwo")
        warm = pc.tile([128, 512], BF, tag="warm")
        nc.gpsimd.memset(negtwo[:], -2.0)
        nc.gpsimd.memset(warm[:], 0.0)
        # preload the Exp activation table during the input-DMA wait so the
        # first real exp doesn't pay the 1.3us table load
        wes = pex.tile([128, 2, 512], BF, tag="es", name="warmes")
        nc.scalar.activation(wes[0:1, 0, 0:1], negtwo[0:1, :], EXP, bias=0.0)

        qkT = [pqk.tile([128, T], BF, tag=f"qk{i}", name=f"qkT{i}") for i in range(4)]
        V = [pvv.tile([128, VW], BF, tag=f"v{i}", name=f"V{i}") for i in range(TT)]
        yT = [pyt.tile([128, T], BF, tag=f"y{i}", name=f"yT{i}") for i in range(2)]
        wqk = pw.tile([128, CT * 512], BF, tag="wqk")
        wv = pw.tile([128, CT * VW], BF, tag="wv")
        wp = pc.tile([128, 2 * C], BF, tag="wp")
        xT = px.tile([128, CT * T], BF, tag="xT")

        # input DMAs: xT is tj-major [tj, ct, 512], wqk is mo-major
        # [mo, ct, 128] so the first qk group only needs 2 DMAs
        nc.sync.dma_start(wqk[:, :1024], d_wqk[:, :1024])
        nc.sync.dma_start(xT[:, :4096], d_xT[:, :4096])
        for mo in range(1, 4):
            nc.sync.dma_start(
                wqk[:, mo * 1024:(mo + 1) * 1024],
                d_wqk[:, mo * 1024:(mo + 1) * 1024])
        nc.sync.dma_start(bqk[:], d_bqk)
        nc.sync.dma_start(wv[:], d_wv)
        nc.sync.dma_start(bv[:], d_bv)
        nc.sync.dma_start(mega[:], d_mega)
        for tj in range(1, TJ):
            nc.sync.dma_start(
                xT[:, tj * 4096:(tj + 1) * 4096],
                d_xT[:, tj * 4096:(tj + 1) * 4096])
        nc.sync.dma_start(wp[:], d_wp)

        # PE p-state warmup during the input-DMA wait (results unused)
        ws = psS.tile([128, 2, 512], F32, tag="s", name="warms")
        for i in range(8):
            nc.tensor.matmul(ws[0:64, 0, :], warm[:, :64], warm[:, :],
                             start=True, stop=True)

        # ---------------- stage A blocks (emitted via filler queue) -------
        def a_qk(tj, mo):
            def emit():
                ps = pfill.tile([128, 512], F32, tag="fill", name=f"psqk{tj}_{mo}")
                for ct in range(CT):
                    nc.tensor.matmul(
                        ps[:],
                        wqk[:, mo * 1024 + ct * 128:mo * 1024 + (ct + 1) * 128],
                        xT[:, tj * 4096 + ct * 512:tj * 4096 + (ct + 1) * 512],
                        start=(ct == 0), stop=(ct == CT - 1))
                nc.vector.tensor_scalar_add(
                    qkT[mo][:, tj * 512:(tj + 1) * 512], ps[:], bqk[:, mo:mo + 1])
            return emit

        def a_v(tt):
            tj, ti = divmod(tt, 4)

            def emit():
                psv = pfill.tile([128, 512], F32, tag="fill", name=f"psv{tt}")
                for ct in range(CT):
                    nc.tensor.matmul(
                        psv[:, :VW],
                        xT[:, tj * 4096 + ct * 512 + ti * 128:
                           tj * 4096 + ct * 512 + (ti + 1) * 128],
                        wv[:, ct * VW:(ct + 1) * VW],
                        start=(ct == 0), stop=(ct == CT - 1))
                nc.vector.tensor_add(V[tt][:], psv[:, :VW], bv[:])
            return emit

        def proj(qj, mo, pool=None, ptag="fill", act_copy=False):
            def emit():
                pps = (pool or pfill).tile([128, 512], F32, tag=ptag,
                                           name=f"pps{qj}_{mo}")
                for kt2 in range(2):
                    nc.tensor.matmul(
                        pps[:],
                        wp[:, kt2 * C + mo * 128:kt2 * C + (mo + 1) * 128],
                        yT[kt2][:, qj * 512:(qj + 1) * 512],
                        start=(kt2 == 0), stop=(kt2 == 1))
                ot = po.tile([128, 512], BF, tag="ot")
                if act_copy:
                    nc.scalar.activation(
                        ot[:], pps[:], mybir.ActivationFunctionType.Copy)
                else:
                    nc.vector.tensor_copy(ot[:], pps[:])
                nc.sync.dma_start(
                    d_out[:, mo, qj * 512:(qj + 1) * 512], ot[:])
            return emit

        # stage A for tj=0 runs up front; the rest interleaves into attention
        for mo in range(4):
            a_qk(0, mo)()
        for tt in range(4):
            a_v(tt)()

        # -------- attention: one global pair pipeline across sections -----
        # units = (qj, hp, lh, p); score-matmuls for unit i+1 are emitted
        # before unit i's exp/AV so the PE never head-of-line blocks on the
        # Act engine, including across lh/hp/qj boundaries.
        units = []
        for qj, hp in [(0, 0), (0, 1), (1, 0), (1, 1),
                       (2, 0), (3, 0), (2, 1), (3, 1)]:
            for lh in range(2):
                for p in range(2 * qj + 2):
                    units.append((qj, hp, lh, p))

        # filler queue: ('A', tj) blocks must drain before any unit of
        # qj=tj and are otherwise held until qj >= tj-1 (don't front-load
        # PE work that later Act-bound windows will starve for); ('P', qj)
        # blocks are appended once yT[qj] is complete.
        fq = []
        for tj in range(1, TJ):
            for mo in range(4):
                fq.append(["A", tj, a_qk(tj, mo), 1707])
            for tt in range(4 * tj, 4 * tj + 4):
                fq.append(["A", tj, a_v(tt), 867])

        # debt-based pacing: filler is emitted only while the PE work emitted
        # so far trails the Act (exp) work emitted so far, so the PE never
        # races ahead and then stalls >3us (which would reset the PE p-state)
        clock = {"pe": 0.0, "act": 0.0}

        def drain_filler(cur_qj, need_tj=None, budget=False, force=0):
            i = 0
            n = 0
            while i < len(fq):
                kind, idx, fn, cost = fq[i][:4]
                forced = need_tj is not None and kind == "A" and idx <= need_tj
                if not forced:
                    if n >= force and (
                            not budget or clock["pe"] >= clock["act"] - 300):
                        break
                    if kind == "A" and idx > cur_qj + 1:
                        i += 1
                        continue
                fn()
                clock["pe"] += cost
                fq.pop(i)
                if not forced:
                    n += 1

        ysb_tiles = {}
        state = {}

        def sc(u):
            qj, hp, lh, p = u
            off = 256 if p == 2 * qj + 1 else 0
            s = psS.tile([128, 2, 512], F32, tag="s")
            es = pex.tile([128, 2, 512], BF, tag="es")
            for ki in range(2):
                kt = 2 * p + ki
                nc.tensor.matmul(
                    s[:, ki, off:512],
                    qkT[2 * hp + 1][64 * lh:64 * lh + 64, kt * 128:(kt + 1) * 128],
                    qkT[2 * hp][64 * lh:64 * lh + 64,
                                qj * 512 + off:(qj + 1) * 512],
                    start=True, stop=True)
            return s, es, off

        # distance to the next qj-crossing, to pre-spread the A-barrier
        nxt = [len(units)] * len(units)
        for i in range(len(units) - 2, -1, -1):
            nxt[i] = i + 1 if units[i + 1][0] != units[i][0] else nxt[i + 1]

        pend = {}
        for i, u in enumerate(units):
            qj, hp, lh, p = u
            npair = 2 * qj + 2
            h_loc = 2 * hp + lh
            if i == 0:
                drain_filler(qj, need_tj=qj)
                pend[0] = sc(u)
            if i + 1 < len(units):
                nqj = units[i + 1][0]
                if nqj > qj:
                    drain_filler(qj, need_tj=nqj)
                pend[i + 1] = sc(units[i + 1])
            s, es, off = pend.pop(i)
            nc.scalar.activation(
                es[:, :, off:512], s[:, :, off:512], EXP, bias=negtwo[:])
            clock["act"] += (1024 - 2 * off) * 0.833 + 185
            clock["pe"] += (512 - off) * 2 * 0.4167
            for ki in range(2):
                kt = 2 * p + ki
                r = kt - 4 * qj
                if r >= 0:
                    # causal mask: keep k<=q <-> p<=c-128r. Only cols
                    # [off, 128(r+1)) can contain k>q; masking just that
                    # range keeps clean AV qt-slices free of the mask dep.
                    mo_ = 384 - 128 * r
                    c1 = 128 * (r + 1)
                    sl = es[:, ki, off:c1]
                    eng = nc.vector if (kt % 2 == 0) else nc.gpsimd
                    eng.tensor_mul(sl, sl, mega[:, mo_ + off:mo_ + c1])
            # filler covers the exp->AV latency, paced by the act/pe debt;
            # near a qj-crossing, force remaining A-blocks out one per unit
            # so the barrier doesn't dump them in a lump
            near_cross = (nxt[i] - i <= 6 and nxt[i] < len(units)
                          and any(e[0] == "A" and e[1] <= units[nxt[i]][0]
                                  for e in fq))
            drain_filler(qj, budget=True,
                         force=1 if (near_cross or (qj, hp) == (3, 1)) else 0)
            # AV (transposed): psy[qt] += es_kt(qt-slice) @ V_kt
            key = (qj, hp, lh)
            if key not in state:
                state[key] = [psY.tile([128, 512], F32, tag="psy",
                                       name=f"psy{qj}_{hp}_{lh}"), True]
            psy, first_mm = state[key]
            for qt in range(4):
                for ki in range(2):
                    kt = 2 * p + ki
                    if kt > 4 * qj + qt:
                        continue
                    nc.tensor.matmul(
                        psy[:, qt * 128:qt * 128 + 65],
                        es[:, ki, qt * 128:(qt + 1) * 128],
                        V[kt][:, h_loc * 65:h_loc * 65 + 65],
                        start=first_mm,
                        stop=(kt == 4 * qj + qt),
                        skip_group_check=True)
                    first_mm = False
                    clock["pe"] += 27
            state[key][1] = first_mm
            if (qj, hp, lh) == (3, 1, 1) and p >= 6:
                # epilogue fast-path: normalize + transpose each qt as soon
                # as its AV accumulation stops so only qt2/qt3 trail the
                # final exp
                qts = (0, 1) if p == 6 else (2, 3)
                y_sb = ysb_tiles[qj]
                rc = pn.tile([128, 4], F32, tag="rc", name=f"rcE{p}")
                for qt in qts:
                    sb_q = pn.tile([128, 128], F32, tag="sbq", name=f"sbq{qt}")
                    nc.vector.tensor_copy(
                        sb_q[:], psy[:, qt * 128:(qt + 1) * 128])
                    nc.vector.reciprocal(rc[:, qt:qt + 1], sb_q[:, 64:65])
                    eng = nc.vector if qt == 3 else nc.gpsimd
                    eng.tensor_scalar_mul(
                        y_sb[qt][:, h_loc * 64:h_loc * 64 + 64],
                        sb_q[:, 0:64], rc[:, qt:qt + 1])
                    nc.sync.dma_start_transpose(
                        yT[1][:, qj * 512 + qt * 128:qj * 512 + (qt + 1) * 128],
                        y_sb[qt][:, 128:256])
                continue
            if p != npair - 1:
                continue
            # last pair of this head: normalize y = psy * (1/l).
            # GPSIMD can't read PSUM: drain psy to SBUF once (DVE), then
            # reciprocal + per-head muls run off SBUF (Pool-legal).
            if qj not in ysb_tiles:
                ysb_tiles[qj] = [
                    pysb.tile([128, 256], BF, tag="ysb", name=f"ysb{qj}_{q}")
                    for q in range(4)]
            y_sb = ysb_tiles[qj]
            sb_y = pn.tile([128, 512], F32, tag="sby")
            nc.vector.tensor_copy(sb_y[:], psy[:])
            rc = pn.tile([128, 4], F32, tag="rc")
            for qt in range(4):
                nc.vector.reciprocal(
                    rc[:, qt:qt + 1], sb_y[:, qt * 128 + 64:qt * 128 + 65])
            for qt in range(4):
                nc.gpsimd.tensor_scalar_mul(
                    y_sb[qt][:, h_loc * 64:h_loc * 64 + 64],
                    sb_y[:, qt * 128:qt * 128 + 64],
                    rc[:, qt:qt + 1])
            if lh == 1:
                # both heads of this pair done: transpose to yT
                for qt in range(4):
                    nc.sync.dma_start_transpose(
                        yT[hp][:, qj * 512 + qt * 128:qj * 512 + (qt + 1) * 128],
                        y_sb[qt][:, hp * 128:(hp + 1) * 128])
                if hp == 1:
                    for mo in range(8):
                        fq.append(["P", qj, proj(qj, mo), 427, mo])
        # drain leftover filler inside the attention scope, then run proj(3)
        # through a wide PSUM ring (psS/psY banks released) so its 8 blocks
        # stream without ring stalls
        while fq:
            fq.pop(0)[2]()
        inner.close()
        # tail proj: copies split DVE/Act, outputs staged into one tile so a
        # single strided DMA replaces 8 serialized HWDGE generations
        with tc.tile_pool(name="tail", bufs=6, space="PSUM") as ptail:
            ot_mega = po.tile([128, 8, 512], BF, tag="otm", name="ot_mega")
            # qt01 columns of yT(3) finish one pair earlier than qt23 (the
            # epilogue transposes them at p==6), so for 6 of 8 mo blocks the
            # first-half matmuls pre-run during the final exp window
            pps_t = {}
            for mo in range(6):
                pps = ptail.tile([128, 512], F32, tag="tp", name=f"tp{mo}")
                pps_t[mo] = pps
                for kt2 in range(2):
                    nc.tensor.matmul(
                        pps[:, 0:256],
                        wp[:, kt2 * C + mo * 128:kt2 * C + (mo + 1) * 128],
                        yT[kt2][:, 3 * 512:3 * 512 + 256],
                        start=(kt2 == 0), stop=(kt2 == 1),
                        skip_group_check=True)
            for mo in range(8):
                if mo < 6:
                    pps = pps_t[mo]
                    for kt2 in range(2):
                        nc.tensor.matmul(
                            pps[:, 256:512],
                            wp[:, kt2 * C + mo * 128:kt2 * C + (mo + 1) * 128],
                            yT[kt2][:, 3 * 512 + 256:4 * 512],
                            start=False, stop=(kt2 == 1),
                            skip_group_check=True)
                else:
                    pps = ptail.tile([128, 512], F32, tag="tp", name=f"tp{mo}")
                    for kt2 in range(2):
                        nc.tensor.matmul(
                            pps[:],
                            wp[:, kt2 * C + mo * 128:kt2 * C + (mo + 1) * 128],
                            yT[kt2][:, 3 * 512:4 * 512],
                            start=(kt2 == 0), stop=(kt2 == 1))
                if mo % 2 == 1:
                    nc.scalar.activation(
                        ot_mega[:, mo, :], pps[:],
                        mybir.ActivationFunctionType.Copy)
                else:
                    nc.vector.tensor_copy(ot_mega[:, mo, :], pps[:])
                # split the final DMA: the bulk streams out while the last
                # two blocks' copies finish, shortening the terminal chain
                if mo == 5:
                    nc.sync.dma_start(
                        d_out[:, 0:6, 3 * 512:4 * 512], ot_mega[:, 0:6, :])
                elif mo == 6:
                    nc.sync.dma_start(
                        d_out[:, 6, 3 * 512:4 * 512], ot_mega[:, 6, :])
            nc.sync.dma_start(
                d_out[:, 7, 3 * 512:4 * 512], ot_mega[:, 7, :])


def _build(reps=1):
    nc = bacc.Bacc("TRN2", target_bir_lowering=False, debug=False)
    d = (
        nc.dram_tensor("xT", [128, CT * T], BF, kind="ExternalInput").ap(),
        nc.dram_tensor("wqk", [128, CT * 512], BF, kind="ExternalInput").ap(),
        nc.dram_tensor("wv", [128, CT * VW], BF, kind="ExternalInput").ap(),
        nc.dram_tensor("wp", [128, 2 * C], BF, kind="ExternalInput").ap(),
        nc.dram_tensor("bqk", [128, 4], F32, kind="ExternalInput").ap(),
        nc.dram_tensor("bv", [128, VW], F32, kind="ExternalInput").ap(),
        nc.dram_tensor("mega", [128, 896], BF, kind="ExternalInput").ap(),
        nc.dram_tensor("outT", [128, 8, T], BF, kind="ExternalOutput").ap(),
    )
    with tile.TileContext(nc) as tc:
        for rep in range(reps):
            if rep:
                tc.strict_bb_all_engine_barrier()
            _emit(tc, nc, d)
    nc.compile()
    return nc


def _sb(a):
    """[128k, n] -> SBUF layout [128, k*n] (k-tile-major along free dim)."""
    k = a.shape[0] // 128
    return np.ascontiguousarray(
        a.reshape(k, 128, a.shape[1]).transpose(1, 0, 2).reshape(128, -1))


def _bf(a):
    return np.ascontiguousarray(a).astype(ml_dtypes.bfloat16)


def _prep_in_maps(inputs):
    x = np.asarray(inputs["x"], np.float32)
    W_attn = np.asarray(inputs["W_attn"], np.float32)
    b_attn = np.asarray(inputs["b_attn"], np.float32)
    W_proj = np.asarray(inputs["W_proj"], np.float32)

    scale = 1.0 / np.sqrt(HD)
    # mega[p, j] = 1 iff j >= p + 384  (causal mask slices)
    mega = (np.arange(896)[None, :] >= np.arange(128)[:, None] + 384)

    in_maps = []
    for c in range(NCORES):
        b, g = divmod(c, 4)
        heads = [4 * g + i for i in range(HPC)]
        xT = _sb(np.ascontiguousarray(x[b].T))                      # [128, 8*2048]
        # tj-major: [128, tj, ct, 512] so one DMA covers a whole tj chunk
        xT = np.ascontiguousarray(
            xT.reshape(128, CT, TJ, 512).transpose(0, 2, 1, 3).reshape(128, -1))

        wq = [W_attn[:, h * HD:(h + 1) * HD] * scale for h in heads]
        wk = [W_attn[:, C + h * HD:C + (h + 1) * HD] for h in heads]
        wqk = np.concatenate(
            [wq[0], wq[1], wk[0], wk[1], wq[2], wq[3], wk[2], wk[3]], axis=1)
        wqk = _sb(wqk)                                              # [128, 8*512]
        # mo-major: [128, mo, ct, 128]
        wqk = np.ascontiguousarray(
            wqk.reshape(128, CT, 4, 128).transpose(0, 2, 1, 3).reshape(128, -1))

        wv = np.zeros((C, VW), np.float32)
        for i, h in enumerate(heads):
            wv[:, i * 65:i * 65 + 64] = W_attn[:, 2 * C + h * HD:2 * C + (h + 1) * HD]
        wv = _sb(wv)                                                # [128, 8*260]

        wp = np.zeros((128, 2 * C), np.float32)
        for kt2 in range(2):
            rows = np.concatenate(
                [W_proj[heads[2 * kt2 + j] * HD:(heads[2 * kt2 + j] + 1) * HD, :]
                 for j in range(2)], axis=0)                        # [128, 1024]
            wp[:, kt2 * C:(kt2 + 1) * C] = rows

        bqk = np.zeros((128, 4), np.float32)
        for i2 in range(2):   # head pair
            for j in range(2):
                h = heads[2 * i2 + j]
                bqk[64 * j:64 * j + 64, 2 * i2] = b_attn[h * HD:(h + 1) * HD] * scale
                bqk[64 * j:64 * j + 64, 2 * i2 + 1] = b_attn[C + h * HD:C + (h + 1) * HD]

        bv = np.zeros(VW, np.float32)
        for i, h in enumerate(heads):
            bv[i * 65:i * 65 + 64] = b_attn[2 * C + h * HD:2 * C + (h + 1) * HD]
            bv[i * 65 + 64] = 1.0
        bv = np.tile(bv[None, :], (128, 1)).astype(np.float32)

        in_maps.append({"xT": _bf(xT), "wqk": _bf(wqk), "wv": _bf(wv),
                        "wp": _bf(wp), "bqk": bqk, "bv": bv,
                        "mega": _bf(mega)})
    return in_maps


def kernel(x, W_attn, b_attn, W_proj, b_proj):
    in_maps = _prep_in_maps(dict(x=x, W_attn=W_attn, b_attn=b_attn,
                                 W_proj=W_proj, b_proj=b_proj))
    if "nc" not in _CACHE:
        _CACHE["nc"] = _build()
    nc = _CACHE["nc"]
    res = run_bass_kernel_spmd(nc, in_maps, core_ids=list(range(NCORES)))

    out = np.zeros((B, T, C), np.float32)
    for c in range(NCORES):
        b = c // 4
        oT = np.asarray(res.results[c]["outT"], np.float32)         # [128, 8*2048]
        oT = oT.reshape(128, 8, T).transpose(1, 0, 2).reshape(C, T)  # [C, T]
        out[b] += oT.T
    out += np.asarray(b_proj, np.float32)[None, None, :]
    return out
